# revision 7
# baseline (speedup 1.0000x reference)
"""GQA attention kernel for Trainium2, 8-core tensor-parallel.

Sharding: 8 cores = 2 batches x 4 KV-groups. Each core handles one
(batch, kv_group): projections for its 4 Q-heads + 1 KV-head, RoPE,
causal attention, and its row-shard of Wo -> partial [T, D] output.
Host sums the 4 partials per batch (the Wo all-reduce) at unshard.

Attention runs in transposed orientation: Q^T/K^T live as [HD, T] so
S^T tiles [s,q] come from single K=128 matmuls; softmax row-sums are
computed by an all-ones [128,128] stationary matmul per s-chunk (which
also broadcasts the sums across partitions); 1/sigma (fast approx
reciprocal) scales each head's O^T before the output projection.
Matmuls run in float32r (~2e-4 rel err at ~233ns per 128x128x512 MM).
"""
from contextlib import ExitStack

import numpy as np

import concourse.bass as bass
import concourse.mybir as mybir
import concourse.tile as tile
from concourse import bacc
from concourse.bass_utils import run_bass_kernel_spmd

B, T, D = 2, 2048, 2048
H, KV, HD = 16, 4, 128
R = H // KV                  # 4 query heads per kv head (per core)
GC = R * HD                  # 512 query-proj cols per core
THETA = 10000.0
TQ = 512                     # q-tile size
NJ = T // TQ                 # 4 q-tiles
ND = D // 128                # 16 contraction chunks
SCALE = float(HD) ** -0.5

F32 = mybir.dt.float32
MM_DT = mybir.dt.float32r
MM_NP = np.float32
BF16 = mybir.dt.bfloat16
AF = mybir.ActivationFunctionType

_CACHED_NC = None


def _build_nc():
    nc = bacc.Bacc("TRN2", target_bir_lowering=False, debug=False, num_devices=8)

    xT = nc.dram_tensor("xT", [D, T], MM_DT, kind="ExternalInput").ap()
    wq = nc.dram_tensor("wq", [D, GC], MM_DT, kind="ExternalInput").ap()
    wk = nc.dram_tensor("wk", [D, HD], MM_DT, kind="ExternalInput").ap()
    wv = nc.dram_tensor("wv", [D, HD], MM_DT, kind="ExternalInput").ap()
    wo = nc.dram_tensor("wo", [GC, D], MM_DT, kind="ExternalInput").ap()
    cosT = nc.dram_tensor("cosT", [HD, T], F32, kind="ExternalInput").ap()
    sinT = nc.dram_tensor("sinT", [HD, T], F32, kind="ExternalInput").ap()
    masks = nc.dram_tensor("masks", [128, 4 * TQ], BF16, kind="ExternalInput").ap()
    out = nc.dram_tensor("out", [T, D], F32, kind="ExternalOutput").ap()

    with tile.TileContext(nc) as tc, ExitStack() as ctx:
        res = ctx.enter_context(tc.tile_pool(name="res", bufs=1))
        sb = ctx.enter_context(tc.tile_pool(name="sb", bufs=2))
        pp = ctx.enter_context(tc.tile_pool(name="pp", bufs=2, space="PSUM"))

        # ---- resident weights / tables ----
        # xt/cos/sin stream on the sync queue; weights go on the scalar /
        # vector engines' queues so j=0's activations aren't stuck behind
        # 10MB of weight DMA.
        wk_sb = res.tile([128, ND * HD], MM_DT)
        nc.sync.dma_start(
            wk_sb[:].rearrange("p (n c) -> p n c", n=ND),
            wk.rearrange("(n p) c -> p n c", p=128),
        )
        wv_sb = res.tile([128, ND * HD], MM_DT)
        nc.sync.dma_start(
            wv_sb[:].rearrange("p (n c) -> p n c", n=ND),
            wv.rearrange("(n p) c -> p n c", p=128),
        )
        # wq chunked so A2 can start as soon as early chunks land
        wq_sb = res.tile([128, ND * GC], MM_DT)    # chunk d at cols [d*GC, (d+1)*GC)
        for d in range(ND):
            nc.scalar.dma_start(wq_sb[:, d * GC:(d + 1) * GC],
                                wq[d * 128:(d + 1) * 128, :])
        mask_sb = res.tile([128, 4 * TQ], BF16)
        nc.gpsimd.dma_start(mask_sb[:], masks[:])
        wo_sb = res.tile([128, R * D], MM_DT)      # head h rows at cols [h*D, (h+1)*D)
        for hh in range(R):
            nc.gpsimd.dma_start(wo_sb[:, hh * D:(hh + 1) * D],
                                wo[hh * 128:(hh + 1) * 128, :])
        kT_sb = res.tile([128, T], MM_DT)          # K^T resident, filled per j
        v_sb = res.tile([128, T], MM_DT)           # V natural, chunk c at cols c*128
        ident = res.tile([128, 128], F32)
        from concourse.masks import make_identity
        make_identity(nc, ident[:])
        ones_f = res.tile([128, 128], F32)
        nc.vector.memset(ones_f[:], 1.0)
        ones_c = res.tile([128, 128], MM_DT)       # sigma-reduce+broadcast stationary
        nc.vector.tensor_copy(ones_c[:], ones_f[:])

        def rope(dst, ps, cosj, sinj):
            # dst = ps * cos + rotate_half(ps) * sin   (partition dim = head dim)
            rot = sb.tile([128, TQ], F32, tag="rot", bufs=2)
            nc.scalar.mul(rot[0:64, :], ps[64:128, :], -1.0)
            nc.scalar.copy(rot[64:128, :], ps[0:64, :])
            tmp = sb.tile([128, TQ], F32, tag="ropetmp", bufs=2)
            nc.vector.tensor_mul(tmp[:], rot[:], sinj[:])
            nc.vector.tensor_mul(dst, ps[:], cosj[:])
            nc.vector.tensor_add(dst, dst.bitcast(F32), tmp[:])

        for j in range(NJ):
            q0 = j * TQ
            # ---- stage inputs for this q/s tile ----
            xts = []
            for d in range(ND):
                xt = sb.tile([128, TQ], MM_DT, tag="xt", bufs=16)
                nc.sync.dma_start(xt[:], xT[d * 128:(d + 1) * 128, q0:q0 + TQ])
                xts.append(xt)
            cosj = sb.tile([128, TQ], F32, tag="cos", bufs=1)
            nc.sync.dma_start(cosj[:], cosT[:, q0:q0 + TQ])
            sinj = sb.tile([128, TQ], F32, tag="sin", bufs=1)
            nc.sync.dma_start(sinj[:], sinT[:, q0:q0 + TQ])

            # ---- A1: K^T and V^T for s-tile j ----
            k_ps = pp.tile([128, TQ], F32, tag="pa", bufs=2)
            vt_ps = pp.tile([128, TQ], F32, tag="pa", bufs=2)
            for d in range(ND):
                nc.tensor.matmul(k_ps[:], wk_sb[:, d * HD:(d + 1) * HD], xts[d][:],
                                 start=(d == 0), stop=(d == ND - 1))
            for d in range(ND):
                nc.tensor.matmul(vt_ps[:], wv_sb[:, d * HD:(d + 1) * HD], xts[d][:],
                                 start=(d == 0), stop=(d == ND - 1))
            rope(kT_sb[:, q0:q0 + TQ], k_ps, cosj, sinj)
            vt_sbt = sb.tile([128, TQ], F32, tag="vtsb", bufs=2)
            nc.vector.tensor_copy(vt_sbt[:], vt_ps[:])
            for c4 in range(4):
                ptt = pp.tile([128, 128], F32, tag="ps", bufs=2)
                nc.tensor.transpose(ptt[:], vt_sbt[:, c4 * 128:(c4 + 1) * 128], ident[:])
                nc.vector.tensor_copy(v_sb[:, (4 * j + c4) * 128:(4 * j + c4 + 1) * 128], ptt[:])

            # ---- A2: Q^T per head + rope ----
            q_tiles = []
            for h in range(R):
                q_ps = pp.tile([128, TQ], F32, tag="pa", bufs=2)
                for d in range(ND):
                    nc.tensor.matmul(
                        q_ps[:], wq_sb[:, d * GC + h * 128:d * GC + (h + 1) * 128],
                        xts[d][:], start=(d == 0), stop=(d == ND - 1))
                qh = sb.tile([128, TQ], MM_DT, tag="qsb", bufs=5)
                rope(qh[:], q_ps, cosj, sinj)
                q_tiles.append(qh)

            # ---- B: causal attention per head ----
            o_tiles = []
            nch = 4 * (j + 1)
            for h in range(R):
                o_ps = pp.tile([128, TQ], F32, tag="po", bufs=2)
                sg_ps = pp.tile([128, TQ], F32, tag="po", bufs=2)
                for c in range(nch):
                    s_ps = pp.tile([128, TQ], F32, tag="ps", bufs=2)
                    nc.tensor.matmul(s_ps[:], kT_sb[:, c * 128:(c + 1) * 128],
                                     q_tiles[h][:], start=True, stop=True)
                    p = sb.tile([128, TQ], MM_DT, tag="psb", bufs=5)
                    nc.scalar.activation(p[:], s_ps[:], AF.Exp, scale=SCALE)
                    if c >= 4 * j:  # diagonal block: apply causal mask
                        m = c - 4 * j
                        nc.vector.tensor_mul(p[:], p[:].bitcast(F32),
                                             mask_sb[:, m * TQ:(m + 1) * TQ])
                    # sigma: ones@p accumulates row-sums broadcast to all parts
                    nc.tensor.matmul(sg_ps[:], ones_c[:], p[:],
                                     start=(c == 0), stop=(c == nch - 1))
                    nc.tensor.matmul(o_ps[:], v_sb[:, c * 128:(c + 1) * 128], p[:],
                                     start=(c == 0), stop=(c == nch - 1))
                sgs = sb.tile([128, TQ], F32, tag="sgs", bufs=2)
                nc.vector.tensor_copy(sgs[:], sg_ps[:])
                rcb = sb.tile([128, TQ], F32, tag="rcb", bufs=2)
                nc.vector.reciprocal_approx_fast(rcb[:], sgs[:])
                oh = sb.tile([128, TQ], MM_DT, tag="osb", bufs=6)
                nc.vector.tensor_mul(oh[:], o_ps[:], rcb[:])
                o_tiles.append(oh)

            # ---- C: output projection for q-tile j ----
            for qs in range(4):
                for n in range(NJ):
                    pc = pp.tile([128, 512], F32, tag="pc", bufs=2)
                    for h in range(R):
                        nc.tensor.matmul(
                            pc[:], o_tiles[h][:, qs * 128:(qs + 1) * 128],
                            wo_sb[:, h * D + n * 512:h * D + (n + 1) * 512],
                            start=(h == 0), stop=(h == R - 1))
                    ob = sb.tile([128, 512], F32, tag="ob", bufs=2)
                    nc.scalar.copy(ob[:], pc[:])
                    nc.gpsimd.dma_start(
                        out[q0 + qs * 128:q0 + (qs + 1) * 128, n * 512:(n + 1) * 512],
                        ob[:])

    nc.compile()
    return nc


def _get_nc():
    global _CACHED_NC
    if _CACHED_NC is None:
        _CACHED_NC = _build_nc()
    return _CACHED_NC


def _rope_tables_T():
    inv_freq = (1.0 / (THETA ** (np.arange(0, HD, 2, dtype=np.float32) / HD))).astype(np.float32)
    pos = np.arange(T, dtype=np.float32)
    freqs = np.outer(pos, inv_freq).astype(np.float32)      # [T, HD/2]
    emb = np.concatenate([freqs, freqs], axis=-1)           # [T, HD]
    return (np.cos(emb).T.astype(np.float32).copy(),
            np.sin(emb).T.astype(np.float32).copy())        # [HD, T]


def _diag_masks():
    # masks[:, m*TQ + jj] for offset delta = m*128: keep jj >= i + delta
    import ml_dtypes
    i = np.arange(128)[:, None]
    jj = np.arange(TQ)[None, :]
    blocks = [(jj >= i + m * 128).astype(ml_dtypes.bfloat16) for m in range(4)]
    return np.concatenate(blocks, axis=1)                   # [128, 4*TQ]


def kernel(x, Wq, Wk, Wv, Wo, _trace=False):
    x = np.asarray(x, dtype=np.float32)
    Wq = np.asarray(Wq, dtype=MM_NP)
    Wk = np.asarray(Wk, dtype=MM_NP)
    Wv = np.asarray(Wv, dtype=MM_NP)
    Wo = np.asarray(Wo, dtype=MM_NP)

    cosT, sinT = _rope_tables_T()
    masks = _diag_masks()
    in_maps = []
    for core in range(8):
        b, g = core // KV, core % KV
        in_maps.append({
            "xT": np.ascontiguousarray(x[b].T.astype(MM_NP)),
            "wq": np.ascontiguousarray(Wq[:, g * GC:(g + 1) * GC]),
            "wk": np.ascontiguousarray(Wk[:, g * HD:(g + 1) * HD]),
            "wv": np.ascontiguousarray(Wv[:, g * HD:(g + 1) * HD]),
            "wo": np.ascontiguousarray(Wo[g * GC:(g + 1) * GC, :]),
            "cosT": cosT, "sinT": sinT, "masks": masks,
        })

    nc = _get_nc()
    res = run_bass_kernel_spmd(nc, in_maps, core_ids=list(range(8)), trace=_trace)

    outp = np.zeros((B, T, D), dtype=np.float32)
    for core in range(8):
        b = core // KV
        outp[b] += res.results[core]["out"]
    if _trace:
        kernel._last_exec_time_ns = res.exec_time_ns
        kernel._last_trace = res.instructions_and_trace
    return outp


# revision 9
# speedup vs baseline: 1.0686x; 1.0686x over previous
"""GQA attention kernel for Trainium2, 8-core tensor-parallel.

Sharding: 8 cores = 2 batches x 4 KV-groups. Each core handles one
(batch, kv_group): projections for its 4 Q-heads + 1 KV-head, RoPE,
causal attention, and its row-shard of Wo -> partial [T, D] output.
Host sums the 4 partials per batch (the Wo all-reduce) at unshard.

Attention runs in transposed orientation: Q^T/K^T live as [HD, T] so
S^T tiles [s,q] come from single K=128 matmuls; softmax row-sums are
computed by an all-ones [128,128] stationary matmul per s-chunk (which
also broadcasts the sums across partitions); 1/sigma (fast approx
reciprocal) scales each head's O^T before the output projection.
Matmuls run in float32r (~2e-4 rel err at ~233ns per 128x128x512 MM).
"""
from contextlib import ExitStack

import numpy as np

import concourse.bass as bass
import concourse.mybir as mybir
import concourse.tile as tile
from concourse import bacc
from concourse.bass_utils import run_bass_kernel_spmd

B, T, D = 2, 2048, 2048
H, KV, HD = 16, 4, 128
R = H // KV                  # 4 query heads per kv head (per core)
GC = R * HD                  # 512 query-proj cols per core
THETA = 10000.0
TQ = 512                     # q-tile size
NJ = T // TQ                 # 4 q-tiles
ND = D // 128                # 16 contraction chunks
SCALE = float(HD) ** -0.5

F32 = mybir.dt.float32
MM_DT = mybir.dt.float32r
MM_NP = np.float32
BF16 = mybir.dt.bfloat16
AF = mybir.ActivationFunctionType

_CACHED_NC = None


def _build_nc():
    nc = bacc.Bacc("TRN2", target_bir_lowering=False, debug=False, num_devices=8)

    xT = nc.dram_tensor("xT", [D, T], MM_DT, kind="ExternalInput").ap()
    wq = nc.dram_tensor("wq", [128, ND * GC], MM_DT, kind="ExternalInput").ap()
    wk = nc.dram_tensor("wk", [128, ND * HD], MM_DT, kind="ExternalInput").ap()
    wv = nc.dram_tensor("wv", [128, ND * HD], MM_DT, kind="ExternalInput").ap()
    wo = nc.dram_tensor("wo", [128, R * D], MM_DT, kind="ExternalInput").ap()
    cosT = nc.dram_tensor("cosT", [HD, T], F32, kind="ExternalInput").ap()
    sinT = nc.dram_tensor("sinT", [HD, T], F32, kind="ExternalInput").ap()
    masks = nc.dram_tensor("masks", [128, 4 * TQ], BF16, kind="ExternalInput").ap()
    out = nc.dram_tensor("out", [T, D], F32, kind="ExternalOutput").ap()

    with tile.TileContext(nc) as tc, ExitStack() as ctx:
        res = ctx.enter_context(tc.tile_pool(name="res", bufs=1))
        sb = ctx.enter_context(tc.tile_pool(name="sb", bufs=2))
        pp = ctx.enter_context(tc.tile_pool(name="pp", bufs=2, space="PSUM"))

        # ---- resident weights / tables ----
        # xt/cos/sin stream on the sync queue; weights go on the scalar /
        # vector engines' queues so j=0's activations aren't stuck behind
        # 10MB of weight DMA.
        # single sync queue, strict priority order: transfers stripe across
        # all 16 DMA engines at full HBM BW, so queue order = arrival order.
        wk_sb = res.tile([128, ND * HD], MM_DT)
        nc.sync.dma_start(wk_sb[:], wk[:])
        xts0 = []
        for d in range(ND):
            xt = sb.tile([128, TQ], MM_DT, tag="xt", bufs=16, name=f"xt0_{d}")
            nc.sync.dma_start(xt[:], xT[d * 128:(d + 1) * 128, 0:TQ])
            xts0.append(xt)
        cosj0 = sb.tile([128, TQ], F32, tag="cos", bufs=1, name="cosj0")
        nc.sync.dma_start(cosj0[:], cosT[:, 0:TQ])
        sinj0 = sb.tile([128, TQ], F32, tag="sin", bufs=1, name="sinj0")
        nc.sync.dma_start(sinj0[:], sinT[:, 0:TQ])
        wv_sb = res.tile([128, ND * HD], MM_DT)
        nc.sync.dma_start(wv_sb[:], wv[:])
        wq_sb = res.tile([128, ND * GC], MM_DT)    # chunk d at cols [d*GC, (d+1)*GC)
        nc.sync.dma_start(wq_sb[:], wq[:])
        mask_sb = res.tile([128, 4 * TQ], BF16)
        nc.sync.dma_start(mask_sb[:], masks[:])
        wo_sb = res.tile([128, R * D], MM_DT)      # head h rows at cols [h*D, (h+1)*D)
        nc.sync.dma_start(wo_sb[:], wo[:])
        kT_sb = res.tile([128, T], MM_DT)          # K^T resident, filled per j
        v_sb = res.tile([128, T], MM_DT)           # V natural, chunk c at cols c*128
        ident = res.tile([128, 128], F32)
        from concourse.masks import make_identity
        make_identity(nc, ident[:])
        ones_f = res.tile([128, 128], F32)
        nc.vector.memset(ones_f[:], 1.0)
        ones_c = res.tile([128, 128], MM_DT)       # sigma-reduce+broadcast stationary
        nc.vector.tensor_copy(ones_c[:], ones_f[:])

        def rope(dst, ps, cosj, sinj):
            # dst = ps * cos + rotate_half(ps) * sin   (partition dim = head dim)
            rot = sb.tile([128, TQ], F32, tag="rot", bufs=2)
            nc.scalar.mul(rot[0:64, :], ps[64:128, :], -1.0)
            nc.scalar.copy(rot[64:128, :], ps[0:64, :])
            tmp = sb.tile([128, TQ], F32, tag="ropetmp", bufs=2)
            nc.vector.tensor_mul(tmp[:], rot[:], sinj[:])
            nc.vector.tensor_mul(dst, ps[:], cosj[:])
            nc.vector.tensor_add(dst, dst.bitcast(F32), tmp[:])

        for j in range(NJ):
            q0 = j * TQ
            # ---- stage inputs for this q/s tile ----
            if j == 0:
                xts, cosj, sinj = xts0, cosj0, sinj0
            else:
                xts = []
                for d in range(ND):
                    xt = sb.tile([128, TQ], MM_DT, tag="xt", bufs=16)
                    nc.sync.dma_start(xt[:], xT[d * 128:(d + 1) * 128, q0:q0 + TQ])
                    xts.append(xt)
                cosj = sb.tile([128, TQ], F32, tag="cos", bufs=1)
                nc.sync.dma_start(cosj[:], cosT[:, q0:q0 + TQ])
                sinj = sb.tile([128, TQ], F32, tag="sin", bufs=1)
                nc.sync.dma_start(sinj[:], sinT[:, q0:q0 + TQ])

            # ---- A1: K^T and V^T for s-tile j ----
            k_ps = pp.tile([128, TQ], F32, tag="pa", bufs=2)
            vt_ps = pp.tile([128, TQ], F32, tag="pa", bufs=2)
            for d in range(ND):
                nc.tensor.matmul(k_ps[:], wk_sb[:, d * HD:(d + 1) * HD], xts[d][:],
                                 start=(d == 0), stop=(d == ND - 1))
            for d in range(ND):
                nc.tensor.matmul(vt_ps[:], wv_sb[:, d * HD:(d + 1) * HD], xts[d][:],
                                 start=(d == 0), stop=(d == ND - 1))
            rope(kT_sb[:, q0:q0 + TQ], k_ps, cosj, sinj)
            vt_sbt = sb.tile([128, TQ], F32, tag="vtsb", bufs=2)
            nc.vector.tensor_copy(vt_sbt[:], vt_ps[:])
            for c4 in range(4):
                ptt = pp.tile([128, 128], F32, tag="pc", bufs=2)
                nc.tensor.transpose(ptt[:], vt_sbt[:, c4 * 128:(c4 + 1) * 128], ident[:])
                nc.vector.tensor_copy(v_sb[:, (4 * j + c4) * 128:(4 * j + c4 + 1) * 128], ptt[:])

            # ---- A2: Q^T per head + rope ----
            q_tiles = []
            for h in range(R):
                q_ps = pp.tile([128, TQ], F32, tag="pa", bufs=2)
                for d in range(ND):
                    nc.tensor.matmul(
                        q_ps[:], wq_sb[:, d * GC + h * 128:d * GC + (h + 1) * 128],
                        xts[d][:], start=(d == 0), stop=(d == ND - 1))
                qh = sb.tile([128, TQ], MM_DT, tag="qsb", bufs=5)
                rope(qh[:], q_ps, cosj, sinj)
                q_tiles.append(qh)

            # ---- B: causal attention per head ----
            o_tiles = []
            nch = 4 * (j + 1)
            for h in range(R):
                o_ps = pp.tile([128, TQ], F32, tag="po", bufs=2)
                sg_ps = pp.tile([128, TQ], F32, tag="po", bufs=2)
                for c in range(nch):
                    s_ps = pp.tile([128, TQ], F32, tag="ps", bufs=2)
                    nc.tensor.matmul(s_ps[:], kT_sb[:, c * 128:(c + 1) * 128],
                                     q_tiles[h][:], start=True, stop=True)
                    p = sb.tile([128, TQ], MM_DT, tag="psb", bufs=5)
                    nc.scalar.activation(p[:], s_ps[:], AF.Exp, scale=SCALE)
                    if c >= 4 * j:  # diagonal block: apply causal mask
                        m = c - 4 * j
                        nc.vector.tensor_mul(p[:], p[:].bitcast(F32),
                                             mask_sb[:, m * TQ:(m + 1) * TQ])
                    # sigma: ones@p accumulates row-sums broadcast to all parts
                    nc.tensor.matmul(sg_ps[:], ones_c[:], p[:],
                                     start=(c == 0), stop=(c == nch - 1))
                    nc.tensor.matmul(o_ps[:], v_sb[:, c * 128:(c + 1) * 128], p[:],
                                     start=(c == 0), stop=(c == nch - 1))
                sgs = sb.tile([128, TQ], F32, tag="sgs", bufs=2)
                nc.vector.tensor_copy(sgs[:], sg_ps[:])
                rcb = sb.tile([128, TQ], F32, tag="rcb", bufs=2)
                nc.vector.reciprocal_approx_fast(rcb[:], sgs[:])
                oh = sb.tile([128, TQ], MM_DT, tag="osb", bufs=6)
                nc.vector.tensor_mul(oh[:], o_ps[:], rcb[:])
                o_tiles.append(oh)

            # ---- C: output projection for q-tile j ----
            for qs in range(4):
                for n in range(NJ):
                    pc = pp.tile([128, 512], F32, tag="pc", bufs=2)
                    for h in range(R):
                        nc.tensor.matmul(
                            pc[:], o_tiles[h][:, qs * 128:(qs + 1) * 128],
                            wo_sb[:, h * D + n * 512:h * D + (n + 1) * 512],
                            start=(h == 0), stop=(h == R - 1))
                    ob = sb.tile([128, 512], F32, tag="ob", bufs=2)
                    nc.scalar.copy(ob[:], pc[:])
                    nc.gpsimd.dma_start(
                        out[q0 + qs * 128:q0 + (qs + 1) * 128, n * 512:(n + 1) * 512],
                        ob[:])

    nc.compile()
    return nc


def _get_nc():
    global _CACHED_NC
    if _CACHED_NC is None:
        _CACHED_NC = _build_nc()
    return _CACHED_NC


def _rope_tables_T():
    inv_freq = (1.0 / (THETA ** (np.arange(0, HD, 2, dtype=np.float32) / HD))).astype(np.float32)
    pos = np.arange(T, dtype=np.float32)
    freqs = np.outer(pos, inv_freq).astype(np.float32)      # [T, HD/2]
    emb = np.concatenate([freqs, freqs], axis=-1)           # [T, HD]
    return (np.cos(emb).T.astype(np.float32).copy(),
            np.sin(emb).T.astype(np.float32).copy())        # [HD, T]


def _diag_masks():
    # masks[:, m*TQ + jj] for offset delta = m*128: keep jj >= i + delta
    import ml_dtypes
    i = np.arange(128)[:, None]
    jj = np.arange(TQ)[None, :]
    blocks = [(jj >= i + m * 128).astype(ml_dtypes.bfloat16) for m in range(4)]
    return np.concatenate(blocks, axis=1)                   # [128, 4*TQ]


def kernel(x, Wq, Wk, Wv, Wo, _trace=False):
    x = np.asarray(x, dtype=np.float32)
    Wq = np.asarray(Wq, dtype=MM_NP)
    Wk = np.asarray(Wk, dtype=MM_NP)
    Wv = np.asarray(Wv, dtype=MM_NP)
    Wo = np.asarray(Wo, dtype=MM_NP)

    cosT, sinT = _rope_tables_T()
    masks = _diag_masks()
    in_maps = []
    for core in range(8):
        b, g = core // KV, core % KV
        def chunkT(w):  # [ND*128, C] -> [128, ND*C] with chunk d at cols [d*C,(d+1)*C)
            nd = w.shape[0] // 128
            return np.ascontiguousarray(
                w.reshape(nd, 128, -1).transpose(1, 0, 2).reshape(128, -1))
        in_maps.append({
            "xT": np.ascontiguousarray(x[b].T.astype(MM_NP)),
            "wq": chunkT(Wq[:, g * GC:(g + 1) * GC]),
            "wk": chunkT(Wk[:, g * HD:(g + 1) * HD]),
            "wv": chunkT(Wv[:, g * HD:(g + 1) * HD]),
            "wo": chunkT(Wo[g * GC:(g + 1) * GC, :]),
            "cosT": cosT, "sinT": sinT, "masks": masks,
        })

    nc = _get_nc()
    res = run_bass_kernel_spmd(nc, in_maps, core_ids=list(range(8)), trace=_trace)

    outp = np.zeros((B, T, D), dtype=np.float32)
    for core in range(8):
        b = core // KV
        outp[b] += res.results[core]["out"]
    if _trace:
        kernel._last_exec_time_ns = res.exec_time_ns
        kernel._last_trace = res.instructions_and_trace
    return outp


# revision 10
# speedup vs baseline: 1.1188x; 1.0470x over previous
"""GQA attention kernel for Trainium2, 8-core tensor-parallel.

Sharding: 8 cores = 2 batches x 4 KV-groups. Each core handles one
(batch, kv_group): projections for its 4 Q-heads + 1 KV-head, RoPE,
causal attention, and its row-shard of Wo -> partial [T, D] output.
Host sums the 4 partials per batch (the Wo all-reduce) at unshard.

Attention runs in transposed orientation: Q^T/K^T live as [HD, T] so
S^T tiles [s,q] come from single K=128 matmuls; softmax row-sums are
computed by an all-ones [128,128] stationary matmul per s-chunk (which
also broadcasts the sums across partitions); 1/sigma (fast approx
reciprocal) scales each head's O^T before the output projection.
Matmuls run in float32r (~2e-4 rel err at ~233ns per 128x128x512 MM).
"""
from contextlib import ExitStack

import numpy as np

import concourse.bass as bass
import concourse.mybir as mybir
import concourse.tile as tile
from concourse import bacc
from concourse.bass_utils import run_bass_kernel_spmd

B, T, D = 2, 2048, 2048
H, KV, HD = 16, 4, 128
R = H // KV                  # 4 query heads per kv head (per core)
GC = R * HD                  # 512 query-proj cols per core
THETA = 10000.0
TQ = 512                     # q-tile size
NJ = T // TQ                 # 4 q-tiles
ND = D // 128                # 16 contraction chunks
SCALE = float(HD) ** -0.5

F32 = mybir.dt.float32
MM_DT = mybir.dt.float32r
MM_NP = np.float32
BF16 = mybir.dt.bfloat16
AF = mybir.ActivationFunctionType

_CACHED_NC = None


def _build_nc():
    nc = bacc.Bacc("TRN2", target_bir_lowering=False, debug=False, num_devices=8)

    xT = nc.dram_tensor("xT", [D, T], MM_DT, kind="ExternalInput").ap()
    wq = nc.dram_tensor("wq", [128, ND * GC], MM_DT, kind="ExternalInput").ap()
    wk = nc.dram_tensor("wk", [128, ND * HD], MM_DT, kind="ExternalInput").ap()
    wv = nc.dram_tensor("wv", [128, ND * HD], MM_DT, kind="ExternalInput").ap()
    wo = nc.dram_tensor("wo", [128, R * D], MM_DT, kind="ExternalInput").ap()
    cosT = nc.dram_tensor("cosT", [HD, T], F32, kind="ExternalInput").ap()
    sinT = nc.dram_tensor("sinT", [HD, T], F32, kind="ExternalInput").ap()
    masks = nc.dram_tensor("masks", [128, 4 * TQ], BF16, kind="ExternalInput").ap()
    out = nc.dram_tensor("out", [T, D], F32, kind="ExternalOutput").ap()

    with tile.TileContext(nc) as tc, ExitStack() as ctx:
        res = ctx.enter_context(tc.tile_pool(name="res", bufs=1))
        sb = ctx.enter_context(tc.tile_pool(name="sb", bufs=2))
        pp = ctx.enter_context(tc.tile_pool(name="pp", bufs=2, space="PSUM"))

        # ---- resident weights / tables ----
        # xt/cos/sin stream on the sync queue; weights go on the scalar /
        # vector engines' queues so j=0's activations aren't stuck behind
        # 10MB of weight DMA.
        # single sync queue, strict priority order: transfers stripe across
        # all 16 DMA engines at full HBM BW, so queue order = arrival order.
        xts0 = []
        for d in range(4):
            xt = sb.tile([128, TQ], MM_DT, tag="xt", bufs=16, name=f"xt0_{d}")
            nc.sync.dma_start(xt[:], xT[d * 128:(d + 1) * 128, 0:TQ])
            xts0.append(xt)
        wk_sb = res.tile([128, ND * HD], MM_DT)
        nc.sync.dma_start(wk_sb[:], wk[:])
        for d in range(4, ND):
            xt = sb.tile([128, TQ], MM_DT, tag="xt", bufs=16, name=f"xt0_{d}")
            nc.sync.dma_start(xt[:], xT[d * 128:(d + 1) * 128, 0:TQ])
            xts0.append(xt)
        cosj0 = sb.tile([128, TQ], F32, tag="cos", bufs=1, name="cosj0")
        nc.sync.dma_start(cosj0[:], cosT[:, 0:TQ])
        sinj0 = sb.tile([128, TQ], F32, tag="sin", bufs=1, name="sinj0")
        nc.sync.dma_start(sinj0[:], sinT[:, 0:TQ])
        wv_sb = res.tile([128, ND * HD], MM_DT)
        nc.sync.dma_start(wv_sb[:], wv[:])
        wq_sb = res.tile([128, ND * GC], MM_DT)    # chunk d at cols [d*GC, (d+1)*GC)
        nc.sync.dma_start(wq_sb[:], wq[:])
        mask_sb = res.tile([128, 4 * TQ], BF16)
        nc.sync.dma_start(mask_sb[:], masks[:])
        wo_sb = res.tile([128, R * D], MM_DT)      # head h rows at cols [h*D, (h+1)*D)
        nc.sync.dma_start(wo_sb[:], wo[:])
        kT_sb = res.tile([128, T], MM_DT)          # K^T resident, filled per j
        v_sb = res.tile([128, T], MM_DT)           # V natural, chunk c at cols c*128
        ident = res.tile([128, 128], F32)
        from concourse.masks import make_identity
        make_identity(nc, ident[:])
        ones_f = res.tile([128, 128], F32)
        nc.vector.memset(ones_f[:], 1.0)
        ones_c = res.tile([128, 128], MM_DT)       # sigma-reduce+broadcast stationary
        nc.vector.tensor_copy(ones_c[:], ones_f[:])

        def rope(dst, ps, cosj, sinj):
            # dst = ps * cos + rotate_half(ps) * sin   (partition dim = head dim)
            rot = sb.tile([128, TQ], F32, tag="rot", bufs=2)
            nc.scalar.mul(rot[0:64, :], ps[64:128, :], -1.0)
            nc.scalar.copy(rot[64:128, :], ps[0:64, :])
            tmp = sb.tile([128, TQ], F32, tag="ropetmp", bufs=2)
            nc.vector.tensor_mul(tmp[:], rot[:], sinj[:])
            nc.vector.tensor_mul(dst, ps[:], cosj[:])
            nc.vector.tensor_add(dst, dst.bitcast(F32), tmp[:])

        for j in range(NJ):
            q0 = j * TQ
            # ---- stage inputs for this q/s tile ----
            if j == 0:
                xts, cosj, sinj = xts0, cosj0, sinj0
            else:
                xts = []
                for d in range(ND):
                    xt = sb.tile([128, TQ], MM_DT, tag="xt", bufs=16)
                    nc.sync.dma_start(xt[:], xT[d * 128:(d + 1) * 128, q0:q0 + TQ])
                    xts.append(xt)
                cosj = sb.tile([128, TQ], F32, tag="cos", bufs=1)
                nc.sync.dma_start(cosj[:], cosT[:, q0:q0 + TQ])
                sinj = sb.tile([128, TQ], F32, tag="sin", bufs=1)
                nc.sync.dma_start(sinj[:], sinT[:, q0:q0 + TQ])

            # ---- A1: K^T and V^T for s-tile j ----
            k_ps = pp.tile([128, TQ], F32, tag="pa", bufs=2)
            vt_ps = pp.tile([128, TQ], F32, tag="pa", bufs=2)
            for d in range(ND):
                nc.tensor.matmul(k_ps[:], wk_sb[:, d * HD:(d + 1) * HD], xts[d][:],
                                 start=(d == 0), stop=(d == ND - 1))
            for d in range(ND):
                nc.tensor.matmul(vt_ps[:], wv_sb[:, d * HD:(d + 1) * HD], xts[d][:],
                                 start=(d == 0), stop=(d == ND - 1))
            rope(kT_sb[:, q0:q0 + TQ], k_ps, cosj, sinj)
            vt_sbt = sb.tile([128, TQ], F32, tag="vtsb", bufs=2)
            nc.vector.tensor_copy(vt_sbt[:], vt_ps[:])
            for c4 in range(4):
                ptt = pp.tile([128, 128], F32, tag="pc", bufs=2)
                nc.tensor.transpose(ptt[:], vt_sbt[:, c4 * 128:(c4 + 1) * 128], ident[:])
                nc.vector.tensor_copy(v_sb[:, (4 * j + c4) * 128:(4 * j + c4 + 1) * 128], ptt[:])

            # ---- A2: Q^T per head + rope ----
            q_tiles = []
            for h in range(R):
                q_ps = pp.tile([128, TQ], F32, tag="pa", bufs=2)
                for d in range(ND):
                    nc.tensor.matmul(
                        q_ps[:], wq_sb[:, d * GC + h * 128:d * GC + (h + 1) * 128],
                        xts[d][:], start=(d == 0), stop=(d == ND - 1))
                qh = sb.tile([128, TQ], MM_DT, tag="qsb", bufs=5)
                rope(qh[:], q_ps, cosj, sinj)
                q_tiles.append(qh)

            # ---- B: causal attention per head ----
            o_tiles = []
            nch = 4 * (j + 1)
            for h in range(R):
                o_ps = pp.tile([128, TQ], F32, tag="po", bufs=2)
                sg_ps = pp.tile([128, TQ], F32, tag="po", bufs=2)
                for c in range(nch):
                    s_ps = pp.tile([128, TQ], F32, tag="ps", bufs=2)
                    nc.tensor.matmul(s_ps[:], kT_sb[:, c * 128:(c + 1) * 128],
                                     q_tiles[h][:], start=True, stop=True)
                    p = sb.tile([128, TQ], MM_DT, tag="psb", bufs=5)
                    nc.scalar.activation(p[:], s_ps[:], AF.Exp, scale=SCALE)
                    if c >= 4 * j:  # diagonal block: apply causal mask
                        m = c - 4 * j
                        nc.vector.tensor_mul(p[:], p[:].bitcast(F32),
                                             mask_sb[:, m * TQ:(m + 1) * TQ])
                    # sigma: ones@p accumulates row-sums broadcast to all parts
                    nc.tensor.matmul(sg_ps[:], ones_c[:], p[:],
                                     start=(c == 0), stop=(c == nch - 1))
                    nc.tensor.matmul(o_ps[:], v_sb[:, c * 128:(c + 1) * 128], p[:],
                                     start=(c == 0), stop=(c == nch - 1))
                sgs = sb.tile([128, TQ], F32, tag="sgs", bufs=2)
                nc.vector.tensor_copy(sgs[:], sg_ps[:])
                rcb = sb.tile([128, TQ], F32, tag="rcb", bufs=2)
                nc.vector.reciprocal_approx_fast(rcb[:], sgs[:])
                oh = sb.tile([128, TQ], MM_DT, tag="osb", bufs=6)
                nc.vector.tensor_mul(oh[:], o_ps[:], rcb[:])
                o_tiles.append(oh)

            # ---- C: output projection for q-tile j ----
            for qs in range(4):
                for n in range(NJ):
                    pc = pp.tile([128, 512], F32, tag="pc", bufs=2)
                    for h in range(R):
                        nc.tensor.matmul(
                            pc[:], o_tiles[h][:, qs * 128:(qs + 1) * 128],
                            wo_sb[:, h * D + n * 512:h * D + (n + 1) * 512],
                            start=(h == 0), stop=(h == R - 1))
                    ob = sb.tile([128, 512], F32, tag="ob", bufs=3)
                    nc.scalar.copy(ob[:], pc[:])
                    nc.gpsimd.dma_start(
                        out[q0 + qs * 128:q0 + (qs + 1) * 128, n * 512:(n + 1) * 512],
                        ob[:])

    nc.compile()
    return nc


def _get_nc():
    global _CACHED_NC
    if _CACHED_NC is None:
        _CACHED_NC = _build_nc()
    return _CACHED_NC


def _rope_tables_T():
    inv_freq = (1.0 / (THETA ** (np.arange(0, HD, 2, dtype=np.float32) / HD))).astype(np.float32)
    pos = np.arange(T, dtype=np.float32)
    freqs = np.outer(pos, inv_freq).astype(np.float32)      # [T, HD/2]
    emb = np.concatenate([freqs, freqs], axis=-1)           # [T, HD]
    return (np.cos(emb).T.astype(np.float32).copy(),
            np.sin(emb).T.astype(np.float32).copy())        # [HD, T]


def _diag_masks():
    # masks[:, m*TQ + jj] for offset delta = m*128: keep jj >= i + delta
    import ml_dtypes
    i = np.arange(128)[:, None]
    jj = np.arange(TQ)[None, :]
    blocks = [(jj >= i + m * 128).astype(ml_dtypes.bfloat16) for m in range(4)]
    return np.concatenate(blocks, axis=1)                   # [128, 4*TQ]


def kernel(x, Wq, Wk, Wv, Wo, _trace=False):
    x = np.asarray(x, dtype=np.float32)
    Wq = np.asarray(Wq, dtype=MM_NP)
    Wk = np.asarray(Wk, dtype=MM_NP)
    Wv = np.asarray(Wv, dtype=MM_NP)
    Wo = np.asarray(Wo, dtype=MM_NP)

    cosT, sinT = _rope_tables_T()
    masks = _diag_masks()
    in_maps = []
    for core in range(8):
        b, g = core // KV, core % KV
        def chunkT(w):  # [ND*128, C] -> [128, ND*C] with chunk d at cols [d*C,(d+1)*C)
            nd = w.shape[0] // 128
            return np.ascontiguousarray(
                w.reshape(nd, 128, -1).transpose(1, 0, 2).reshape(128, -1))
        in_maps.append({
            "xT": np.ascontiguousarray(x[b].T.astype(MM_NP)),
            "wq": chunkT(Wq[:, g * GC:(g + 1) * GC]),
            "wk": chunkT(Wk[:, g * HD:(g + 1) * HD]),
            "wv": chunkT(Wv[:, g * HD:(g + 1) * HD]),
            "wo": chunkT(Wo[g * GC:(g + 1) * GC, :]),
            "cosT": cosT, "sinT": sinT, "masks": masks,
        })

    nc = _get_nc()
    res = run_bass_kernel_spmd(nc, in_maps, core_ids=list(range(8)), trace=_trace)

    outp = np.zeros((B, T, D), dtype=np.float32)
    for core in range(8):
        b = core // KV
        outp[b] += res.results[core]["out"]
    if _trace:
        kernel._last_exec_time_ns = res.exec_time_ns
        kernel._last_trace = res.instructions_and_trace
    return outp
